# revision 33
# baseline (speedup 1.0000x reference)
"""BitConv2dInfer on 8 Trainium2 NeuronCores — fp8 DoubleRow version.

Reference computation (per full input):
    x = clip(x, -1, 1)                       # x [32, 256, 56, 56] f32
    y = conv2d(x, w_q, pad=1)                # w_q [256, 256, 3, 3] ternary
    y = y * s + bias                         # per-out-channel affine
Sharding: data-parallel over batch — each of the 8 cores gets 4 images and
the full (tiny) weights; outputs concatenate over batch with no comms.

Numerics: x is host-cast to bf16 for transport and quantized to fp8 e4m3 on
device (after the clamp). Ternary weights are exact in e4m3. Measured
end-to-end rel err vs the f32 reference is 1.434e-2 (gate 2e-2) on the fixed
key-0 inputs. The payoff: the PE runs e4m3 matmuls in DoubleRow mode — both
128-deep cin halves contract in one instruction (virtual K=256, 2 fp8
mults/cell/cycle), halving PE streaming time vs bf16 (~190ns per 448-column
matmul at 2.4 GHz).

Device kernel (per core, per image):
  - DMA x[n] in as 2 CIN tiles of [128, 56, 56] bf16 (row-chunked for the
    first image so the PE can start before the full image lands; steady-state
    images issue from the gpsimd queue, which never blocks behind PSUM-gated
    ACTIVATEs the way the scalar/sync queues do)
  - clamp to [-1,1] + cast to e4m3 into a zero-bordered [128, 2, 58, 64]
    pad tile (64-wide rows keep the ci-plane stride 16B-aligned for the
    DoubleRow weight/ifmap AP rules; cols 58-63 are never read)
  - conv as 9 accumulated DoubleRow PE matmuls per (cout_tile, 8-row chunk):
      psum[co*128+m, oh, ow] += sum_ci sum_k w[k, ci, m] * xpad[k, ci, oh+kh, ow+kw]
    lhsT = w slice [128, 2, 128], rhs = shifted pad-tile window [128, 2, 8, 56]
  - image 0 walks groups outermost (co inner) so each input row chunk feeds
    2 cout tiles of PE work before the next chunk is needed
  - scalar-engine activation evacuates PSUM with per-partition scale+bias
  - DMA f32 result tiles back out, split across the sync and scalar rings;
    the last image streams each group out as soon as its ACT lands,
    alternating sync/gpsimd rings, with tapered closing groups so the tail
    drains fast

The PE clock gate (HAM) starts at 1.2 GHz and only reaches 2.4 GHz after
~3.4us of sustained activity, so the kernel front-runs dummy matmuls on a
zeroed tile while the first input chunks are in flight.
"""

import sys

sys.path.insert(0, "/opt/trn_rl_repo")

import ml_dtypes
import numpy as np

import concourse.bass as bass  # noqa: F401  (registers engines)
import concourse.mybir as mybir
import concourse.tile as tile
from concourse import bacc
from concourse.bass_utils import run_bass_kernel_spmd

N, CIN, COUT, H, W = 32, 256, 256, 56, 56
NCORES = 8
NB = N // NCORES          # images per core
HP = H + 2                # padded rows
WPP = 64                  # padded row pitch (56+2 used; 64 keeps plane 16B-aligned)
RG = 8                    # output rows per PSUM chunk (8*56=448 <= 512 f32/bank)
NCH = H // RG             # chunks per image
NCI = CIN // 128          # cin tiles
NCO = COUT // 128         # cout tiles
NTAP = 9
# First-image input chunk schedule, (engine, ci, row0, nrows) in issue order.
# Image 0 runs group-outer/co-inner: group g (8 rows, both couts) needs input
# rows [8g-1, 8g+9). x rides as bf16 (host-cast), so each ring delivers
# ~6-10 rows/us and every chunk beats its ~3.4us-period group deadline.
N0_CHUNKS = [
    ("g", 0, 0, 9), ("s", 1, 0, 9),
    ("g", 0, 9, 16), ("s", 1, 9, 16),
    ("g", 0, 25, 31), ("s", 1, 25, 31),
]
N_WARM_MM = 17            # dummy matmuls to lift the HAM clock gate

_compiled = {}


def _build():
    nc = bacc.Bacc("TRN2", target_bir_lowering=False, debug=False)
    f32, bf16, fp8 = mybir.dt.float32, mybir.dt.bfloat16, mybir.dt.float8e4
    DR = mybir.MatmulPerfMode.DoubleRow
    x_d = nc.dram_tensor("x", [NB, CIN, H, W], bf16, kind="ExternalInput").ap()
    w_d = nc.dram_tensor(
        "w", [128, NCO, NTAP, NCI, 128], fp8, kind="ExternalInput"
    ).ap()
    sb_d = nc.dram_tensor("sb", [128, 2 * NCO], f32, kind="ExternalInput").ap()
    o_d = nc.dram_tensor("out", [NB, COUT, H, W], bf16, kind="ExternalOutput").ap()

    clamp = dict(op0=mybir.AluOpType.max, op1=mybir.AluOpType.min)

    with tile.TileContext(nc) as tc:
        with (
            tc.tile_pool(name="const", bufs=1) as cpool,
            tc.tile_pool(name="xs", bufs=4) as xspool,
            tc.tile_pool(name="xsc", bufs=3) as xscpool,
            tc.tile_pool(name="xpad", bufs=3) as xppool,
            tc.tile_pool(name="osb", bufs=3) as opool,
            tc.tile_pool(name="ps", bufs=6, space="PSUM") as pspool,
            tc.tile_pool(name="warmps", bufs=1, space="PSUM") as wpspool,
        ):
            w_sb = cpool.tile([128, NCO, NTAP, NCI, 128], fp8, tag="w")
            sb_sb = cpool.tile([128, 2 * NCO], f32, tag="sb")

            # HAM pre-warm. The memset is the first vector instruction so the
            # warm matmuls can start the PE clock ramp almost immediately.
            warm = cpool.tile([128, RG * W], fp8, tag="warm")
            nc.vector.memset(warm[:], 0.0)
            warm_ps = wpspool.tile([128, RG * W], f32, tag="warmps")
            for _ in range(N_WARM_MM):
                nc.tensor.matmul(
                    out=warm_ps[:], lhsT=warm[:, 0:128], rhs=warm[:],
                    start=True, stop=True,
                )

            def fresh_xpad():
                xpad = xppool.tile([128, NCI, HP, WPP], fp8, tag="xpad")
                for ci in range(NCI):
                    nc.vector.memset(xpad[:, ci, 0:1, 0:58], 0.0)
                    nc.vector.memset(xpad[:, ci, HP - 1:HP, 0:58], 0.0)
                    nc.vector.memset(xpad[:, ci, 1:HP - 1, 0:1], 0.0)
                    nc.vector.memset(xpad[:, ci, 1:HP - 1, 57:58], 0.0)
                return xpad

            # Weights first on the idle sync ring: co0 gates the very first
            # matmul (~12.7us), co1 lands just before group 0's co1 pass.
            # (Starting earlier via tap-pieced weights or finer head chunks
            # was tried and LOSES: all downstream chunk deadlines cascade
            # from the first matmul, so a later-but-comfortable start keeps
            # the PE gapless while an early start exposes arrival jitter.)
            nc.sync.dma_start(out=w_sb[:, 0], in_=w_d[:, 0])
            nc.sync.dma_start(out=w_sb[:, 1], in_=w_d[:, 1])

            # First image, row-chunked: ci0 on the gpsimd ring, ci1 on the
            # scalar ring, so the 9-row heads land in parallel by ~12us.
            xp0 = fresh_xpad()
            n0_stage = []
            for k, (eng_key, ci, r0, nr) in enumerate(N0_CHUNKS):
                eng = nc.gpsimd if eng_key == "g" else nc.scalar
                xs = xscpool.tile([128, 31, W], bf16, tag=f"xsc{ci}")
                eng.dma_start(
                    out=xs[:, 0:nr],
                    in_=x_d[0, ci * 128:(ci + 1) * 128, r0:r0 + nr],
                )
                n0_stage.append((r0, nr, ci, xs))
                if k == 1:
                    nc.gpsimd.dma_start(out=sb_sb[:], in_=sb_d)
            for r0, nr, ci, xs in n0_stage:
                nc.vector.tensor_scalar(
                    xp0[:, ci, r0 + 1:r0 + nr + 1, 1:W + 1],
                    xs[:, 0:nr], -1.0, 1.0, **clamp,
                )

            def conv_group(xpad, n, co, g0, gn, osb, stream_eng):
                ps = pspool.tile([128, RG, W], f32, tag="ps")
                for t in range(NTAP):
                    kh, kw = divmod(t, 3)
                    nc.tensor.matmul(
                        out=ps[:, 0:gn],
                        lhsT=w_sb[:, co, t],
                        rhs=xpad[:, :, g0 + kh:g0 + kh + gn, kw:kw + W],
                        start=(t == 0),
                        stop=(t == NTAP - 1),
                        perf_mode=DR,
                    )
                nc.scalar.activation(
                    out=osb[:, g0:g0 + gn, :], in_=ps[:, 0:gn],
                    func=mybir.ActivationFunctionType.Identity,
                    bias=sb_sb[:, NCO + co:NCO + co + 1],
                    scale=sb_sb[:, co:co + 1],
                )
                if stream_eng is not None:
                    stream_eng.dma_start(
                        out=o_d[n, co * 128:(co + 1) * 128, g0:g0 + gn],
                        in_=osb[:, g0:g0 + gn],
                    )

            for n in range(NB):
                if n == 0:
                    xp = xp0
                else:
                    # Steady-state ci DMAs issue from gpsimd: its queue has no
                    # compute instructions, so the issues never serialize
                    # behind PSUM-gated ACTIVATEs (scalar) or output DMAs
                    # (sync) of the previous image.
                    xp = fresh_xpad()
                    for ci in range(NCI):
                        xs = xspool.tile([128, H, W], bf16, tag="xs")
                        nc.gpsimd.dma_start(
                            out=xs[:], in_=x_d[n, ci * 128:(ci + 1) * 128]
                        )
                        nc.vector.tensor_scalar(
                            xp[:, ci, 1:H + 1, 1:W + 1], xs[:], -1.0, 1.0, **clamp
                        )
                # bf16 result tiles: halves output HBM traffic (the e4m3 conv
                # noise dwarfs the extra 2^-9 rounding; host upcasts to f32).
                osb = [
                    opool.tile([128, H, W], bf16, tag="osb", name=f"osb{i}")
                    for i in range(NCO)
                ]
                last_img = n == NB - 1
                if n == 0:
                    # Group-outer, co-inner: each input chunk feeds 2x PE work,
                    # relaxing the first-image DMA chunk deadlines.
                    for c in range(NCH):
                        for co in range(NCO):
                            conv_group(xp, n, co, c * RG, RG, osb[co], None)
                elif last_img:
                    # Last image: stream every group out as soon as its ACT
                    # lands, alternating rings. The final group's (small,
                    # bf16) transfer splits across both rings so the tail
                    # drains in ~0.4us. No tapered sub-groups: 112-col
                    # DoubleRow matmuls pay the FD<128 penalty, costing more
                    # PE time than the finer drain saves.
                    rings = [nc.sync, nc.gpsimd]
                    for co in range(NCO):
                        for c in range(NCH):
                            final = co == NCO - 1 and c == NCH - 1
                            conv_group(xp, n, co, c * RG, RG, osb[co],
                                       None if final else rings[c % 2])
                    dst = o_d[n, 128:256, H - RG:H]
                    src = osb[NCO - 1][:, H - RG:H]
                    nc.sync.dma_start(out=dst[:, 0:4], in_=src[:, 0:4])
                    nc.gpsimd.dma_start(out=dst[:, 4:RG], in_=src[:, 4:RG])
                else:
                    for co in range(NCO):
                        for c in range(NCH):
                            conv_group(xp, n, co, c * RG, RG, osb[co], None)
                if not last_img:
                    for co in range(NCO):
                        dst = o_d[n, co * 128:(co + 1) * 128]
                        nc.sync.dma_start(out=dst[:, 0:36], in_=osb[co][:, 0:36])
                        nc.scalar.dma_start(out=dst[:, 36:H], in_=osb[co][:, 36:H])

    nc.compile()
    return nc


def _prep_weights(w_q, s, bias):
    # lhsT layout: [cin_k (128 partitions), co, tap, ci, cout_j] so that
    # w_t[k, co, t, ci, j] = w_q[co*128 + j, ci*128 + k, kh, kw]
    w_t = (
        w_q.astype(np.float32)
        .reshape(NCO, 128, NCI, 128, 3, 3)     # [co, j, ci, k, kh, kw]
        .transpose(3, 0, 4, 5, 2, 1)           # [k, co, kh, kw, ci, j]
        .reshape(128, NCO, NTAP, NCI, 128)
        .astype(mybir.dt.np(mybir.dt.float8e4))
    )
    sb_t = np.concatenate(
        [
            np.ascontiguousarray(s.reshape(NCO, 128).T.astype(np.float32)),
            np.ascontiguousarray(bias.reshape(NCO, 128).T.astype(np.float32)),
        ],
        axis=1,
    )
    return np.ascontiguousarray(w_t), np.ascontiguousarray(sb_t)


def kernel(x, w_q, s, bias):
    if "nc" not in _compiled:
        _compiled["nc"] = _build()
    nc = _compiled["nc"]

    w_t, sb_t = _prep_weights(w_q, s, bias)
    x = np.ascontiguousarray(
        np.asarray(x, dtype=np.float32).astype(ml_dtypes.bfloat16)
    )
    core_ids = list(range(NCORES))
    in_maps = [
        {"x": x[i * NB:(i + 1) * NB], "w": w_t, "sb": sb_t}
        for i in core_ids
    ]
    res = run_bass_kernel_spmd(nc, in_maps, core_ids)
    out = np.concatenate([res.results[i]["out"] for i in core_ids], axis=0)
    return out.astype(np.float32)
